# revision 38
# baseline (speedup 1.0000x reference)
"""Trainium2 Bass kernel for nn_Cca3 channel cross-attention.

Reference computation (per pair b of 8):
  x_s, x_t : [128, N] (N = 128*128 spatial), C = 128 channels
  q/k/v = 1x1 conv projections (w @ x + b) of both streams
  S1 = q_t @ k_s^T  (contract over N) -> a_st = rowsoftmax(S1)
  S2 = q_s @ k_t^T                    -> a_ts = rowsoftmax(S2)
  att = rowsoftmax(a_st @ a_ts^T)
  out_s = x_s + att @ v_s ; out_t = x_t + att @ v_t

Key algebra (projections are linear => collapse everything):
  G  = X_t @ X_s^T   (accumulated over n in 128-col chunks, fp16 inputs)
  S1 = qw G kw^T   + outer(qb, kw rs_s + N kb) + outer(qw rs_t, kb)
  S2 = qw G^T kw^T + outer(qb, kw rs_t + N kb) + outer(qw rs_s, kb)
       where rs_s = X_s @ 1, rs_t = X_t @ 1 (row sums, free via 1-col matmuls)
  out_{s,t} = (att vw + I) @ X_{s,t} + (att vb) . 1^T
       i.e. phase 2 is a single 1x1 conv with weights A = att vw + I.

Sharding: data-parallel, one (x_s[i], x_t[i]) pair per NeuronCore (8 cores).
All I/O in fp16 (inputs cast on host; outputs upcast on host): halves HBM
traffic vs fp32 and runs the PE at 1 cycle/row.

Per-core pipeline:
  - DMA x in 1024-col fp16 slabs, both streams interleaved.
  - Phase 1 per 128-col chunk: PE-transpose xs/xt chunks to PSUM (fp16),
    pack 4 chunks per PSUM tile, one DVE/ACT copy per pack into an SBUF
    ring; then G += Tt_j^T Ts_j plus two 1-col row-sum matmuls per chunk.
    Emitted one group behind the transposes so PE never waits on copies.
  - Tail: S1/S2 via two small matmul chains + rank-1 bias corrections,
    three softmaxes (final one needs no max; scores do), att composition
    via PE transposes, A^T = vw^T att^T + I, c = att vb.
  - Phase 2 per 512-col chunk: out = A^T^T X (one matmul), biased
    copy/cast PSUM->SBUF fp16 (DVE/ACT alternating), DMA out.
"""

from contextlib import ExitStack

import numpy as np

C = 128
N_FULL = 16384
SLABS = [512, 512, 1024, 2048, 4096, 4096, 4096]  # input DMA slab widths
GRP = 4      # chunks per transpose-pack / copy (one PSUM bank)
RING = 4     # SBUF transpose ring depth, in groups
F2 = 512     # phase-2 matmul chunk width (one PSUM bank)
HP = 1024    # phase-2 psum tile width (2 banks, 2 matmuls, 1 copy)
STG = 4096   # phase-2 output staging width (one DMA per stage)


def build_nc(n=N_FULL):
    import concourse.bacc as bacc
    import concourse.tile as tile
    from concourse import mybir
    from concourse.masks import make_identity

    f32 = mybir.dt.float32
    f16 = mybir.dt.float16
    AF = mybir.ActivationFunctionType
    AX = mybir.AxisListType

    assert sum(SLABS) == n
    nchunks = n // C
    ngroups = nchunks // GRP

    nc = bacc.Bacc("TRN2", target_bir_lowering=False, debug=False)

    def din(name, shape, dt=f16):
        return nc.dram_tensor(name, shape, dt, kind="ExternalInput").ap()

    def dout(name, shape):
        return nc.dram_tensor(name, shape, f16, kind="ExternalOutput").ap()

    xs_d = din("xs", [C, n])
    xt_d = din("xt", [C, n])
    wqk_d = din("wqk", [C, 2 * C])  # [qw.T | kw.T]
    vw_d = din("vw_p", [C, C])      # plain vw (row-major, partition=row)
    qb_d = din("qb_row", [1, C])
    kb_d = din("kb_row", [1, C])
    kbN_d = din("kbN_row", [1, C])  # n * kb
    vb_d = din("vb_col", [C, 1])
    ys_d = dout("ys", [C, n])
    yt_d = dout("yt", [C, n])

    with tile.TileContext(nc) as tc, ExitStack() as ctx:
        singles = ctx.enter_context(tc.tile_pool(name="singles", bufs=1))

        # ---- persistent SBUF ----
        xs_sb = singles.tile([C, n], f16, tag="xs")
        xt_sb = singles.tile([C, n], f16, tag="xt")
        wqk_sb = singles.tile([C, 2 * C], f16, tag="wqk")
        vw_sb = singles.tile([C, C], f16, tag="vw")
        ident_sb = singles.tile([C, C], f16, tag="ident")
        ones_sb = singles.tile([C, 1], f16, tag="ones")
        qb_sb = singles.tile([1, C], f16, tag="qb")
        kb_sb = singles.tile([1, C], f16, tag="kb")
        kbN_sb = singles.tile([1, C], f16, tag="kbN")
        vb_sb = singles.tile([C, 1], f16, tag="vb")
        tring = singles.tile([C, RING * 2 * GRP * C], f16, tag="tring")
        warm_sb = singles.tile([1, 2], f32, tag="warm")

        make_identity(nc, ident_sb)
        # warm the ACT exp table early (overlaps input DMA)
        nc.vector.memset(warm_sb, 0.0)
        nc.scalar.activation(out=warm_sb, in_=warm_sb, func=AF.Exp)

        # ---- input DMA ----
        # Small slabs first for a fast pipeline start, then big ones to
        # amortize per-DMA overheads; small weights ride the SP queue's
        # slack right after the first slab pair (wire cost ~0.5us).
        def span_dma(lo, hi):
            # two DMA queues (SP hardware DGE, Pool software DGE) so the
            # xs/xt halves of each span interleave on the wire
            sl = slice(lo, hi)
            nc.sync.dma_start(out=xs_sb[:, sl], in_=xs_d[:, sl])
            nc.gpsimd.dma_start(out=xt_sb[:, sl], in_=xt_d[:, sl])

        nc.vector.memset(ones_sb, 1.0)
        lo = 0
        for w in SLABS:
            span_dma(lo, lo + w)
            lo += w
        # small weights (needed only by the tail) ride after the slabs
        nc.sync.dma_start(out=wqk_sb, in_=wqk_d)
        nc.sync.dma_start(out=vw_sb, in_=vw_d)
        nc.sync.dma_start(out=qb_sb, in_=qb_d)
        nc.sync.dma_start(out=kb_sb, in_=kb_d)
        nc.sync.dma_start(out=kbN_sb, in_=kbN_d)
        nc.sync.dma_start(out=vb_sb, in_=vb_d)

        # ---- tail smalls (SBUF) ----
        smalls = ctx.enter_context(tc.tile_pool(name="smalls", bufs=1))
        G_sb = smalls.tile([C, C], f16, tag="G")
        GT_sb = smalls.tile([C, C], f16, tag="GT")
        U1_sb = smalls.tile([C, C], f16, tag="U1")
        U2_sb = smalls.tile([C, C], f16, tag="U2")
        rss_sb = smalls.tile([C, 1], f16, tag="rss")
        rst_sb = smalls.tile([C, 1], f16, tag="rst")
        sums1_sb = smalls.tile([1, 2 * C], f16, tag="sums1")  # [Q0s|K0s]
        sums2_sb = smalls.tile([1, 2 * C], f16, tag="sums2")  # [Q0t|K0t]
        cks_sb = smalls.tile([1, C], f16, tag="cks")
        ckt_sb = smalls.tile([1, C], f16, tag="ckt")
        ast_sb = smalls.tile([C, C], f16, tag="ast")
        ats_sb = smalls.tile([C, C], f16, tag="ats")
        att_sb = smalls.tile([C, C], f16, tag="att")
        astT_sb = smalls.tile([C, C], f16, tag="astT")
        atsT_sb = smalls.tile([C, C], f16, tag="atsT")
        attT_sb = smalls.tile([C, C], f16, tag="attT")
        AT_sb = smalls.tile([C, C], f16, tag="AT")
        c_sb = smalls.tile([C, 1], f32, tag="catt")

        # =========================== phase 1 ===========================
        with tc.tile_pool(name="gps", bufs=1, space="PSUM") as g_pool, \
             tc.tile_pool(name="rsps", bufs=1, space="PSUM") as rs_pool:
            G_ps = g_pool.tile([C, C], f32, tag="G")
            rss_ps = rs_pool.tile([C, 1], f32, tag="rss")
            rst_ps = rs_pool.tile([C, 1], f32, tag="rst")

            with tc.tile_pool(name="tps", bufs=4, space="PSUM") as t_pool:

                # slot layout matches the PSUM pack: [Ts-pack | Tt-pack];
                # the copy split is column-balanced for engine speed
                # (DVE 2x mode ~0.52ns/col vs ACT ~0.83ns/col)
                W = 2 * GRP * C
                CUT = 5 * GRP * C // 4

                def emit_group_transposes(g):
                    tp = t_pool.tile([C, W], f16, tag="tp")
                    for j in range(GRP):
                        ch = g * GRP + j
                        sl = slice(ch * C, (ch + 1) * C)
                        blS = slice(j * C, (j + 1) * C)
                        blT = slice((GRP + j) * C, (GRP + j + 1) * C)
                        nc.tensor.transpose(tp[:, blS], xs_sb[:, sl], ident_sb)
                        nc.tensor.transpose(tp[:, blT], xt_sb[:, sl], ident_sb)
                    st = (g % RING) * W
                    nc.vector.tensor_copy(tring[:, st:st + CUT], tp[:, 0:CUT])
                    nc.scalar.copy(tring[:, st + CUT:st + W], tp[:, CUT:W])

                def emit_group_matmuls(g):
                    st = (g % RING) * W
                    for j in range(GRP):
                        ch = g * GRP + j
                        first, last = (ch == 0), (ch == nchunks - 1)
                        Ts_j = tring[:, st + j * C:st + (j + 1) * C]
                        Tt_j = tring[:, st + (GRP + j) * C:
                                     st + (GRP + j + 1) * C]
                        nc.tensor.matmul(G_ps, lhsT=Tt_j, rhs=Ts_j,
                                         start=first, stop=last)
                        nc.tensor.matmul(rss_ps, lhsT=Ts_j, rhs=ones_sb,
                                         start=first, stop=last)
                        nc.tensor.matmul(rst_ps, lhsT=Tt_j, rhs=ones_sb,
                                         start=first, stop=last)

                PIPE_G = 2  # G-matmuls trail transposes by this many groups
                for g in range(ngroups + PIPE_G):
                    if g < ngroups:
                        emit_group_transposes(g)
                    if g >= PIPE_G:
                        emit_group_matmuls(g - PIPE_G)

            # ======================== tail =========================
            # Five PSUM banks, hand-packed by lifetime:
            #  t1: s1r -> S1 -> astT -> AT ; t2: s2r -> S2 -> atsT -> c
            #  t3: GT -> m ; t4: U1 -> attT ; t5: U2
            with tc.tile_pool(name="tailps", bufs=1, space="PSUM") as tl:
                # G / GT / row sums to SBUF
                nc.vector.tensor_copy(G_sb, G_ps)
                nc.vector.tensor_copy(rss_sb, rss_ps)
                nc.vector.tensor_copy(rst_sb, rst_ps)
                GT_ps = tl.tile([C, C], f16, tag="t3")
                nc.tensor.transpose(GT_ps, G_sb, ident_sb)
                nc.scalar.copy(GT_sb, GT_ps)

                # [Q0|K0] rows from row sums
                sums1_ps = tl.tile([1, 2 * C], f32, tag="t1")
                sums2_ps = tl.tile([1, 2 * C], f32, tag="t2")
                nc.tensor.matmul(sums1_ps, lhsT=rss_sb, rhs=wqk_sb,
                                 start=True, stop=True)
                nc.tensor.matmul(sums2_ps, lhsT=rst_sb, rhs=wqk_sb,
                                 start=True, stop=True)
                nc.vector.tensor_copy(sums1_sb, sums1_ps)
                nc.vector.tensor_copy(sums2_sb, sums2_ps)
                nc.vector.tensor_add(cks_sb, sums1_sb[0:1, C:2 * C], kbN_sb)
                nc.vector.tensor_add(ckt_sb, sums2_sb[0:1, C:2 * C], kbN_sb)

                # U2 first: the S2 branch is longer (GT detour), so its
                # ops lead on every engine. U1 = (qw G)^T, U2 = (qw G^T)^T
                U1_ps = tl.tile([C, C], f32, tag="t4")
                U2_ps = tl.tile([C, C], f32, tag="t5")
                nc.tensor.matmul(U2_ps, lhsT=GT_sb, rhs=wqk_sb[:, 0:C],
                                 start=True, stop=True)
                nc.scalar.copy(U2_sb, U2_ps)
                nc.tensor.matmul(U1_ps, lhsT=G_sb, rhs=wqk_sb[:, 0:C],
                                 start=True, stop=True)
                nc.vector.tensor_copy(U1_sb, U1_ps)

                # S2 / S1 with rank-1 bias corrections accumulated in PSUM
                S1_ps = tl.tile([C, C], f32, tag="t1")
                S2_ps = tl.tile([C, C], f32, tag="t2")
                nc.tensor.matmul(S2_ps, lhsT=U2_sb, rhs=wqk_sb[:, C:2 * C],
                                 start=True, stop=True)
                nc.tensor.matmul(S2_ps, lhsT=qb_sb, rhs=ckt_sb,
                                 start=False, stop=False, skip_group_check=True)
                nc.tensor.matmul(S2_ps, lhsT=sums1_sb[0:1, 0:C], rhs=kb_sb,
                                 start=False, stop=True, skip_group_check=True)
                nc.tensor.matmul(S1_ps, lhsT=U1_sb, rhs=wqk_sb[:, C:2 * C],
                                 start=True, stop=True)
                nc.tensor.matmul(S1_ps, lhsT=qb_sb, rhs=cks_sb,
                                 start=False, stop=False, skip_group_check=True)
                nc.tensor.matmul(S1_ps, lhsT=sums2_sb[0:1, 0:C], rhs=kb_sb,
                                 start=False, stop=True, skip_group_check=True)

                # Row softmaxes. The a_st branch stays UNnormalized (E1);
                # its 1/rowsum folds into the final exp's per-row scale:
                #   m = diag(r1) E1 a_ts^T ; att = rowsoftmax(m)
                # and since m's entries are bounded, no max-shift needed.
                def rowexp(src_ps, dst, tg):
                    ssum = smalls.tile([C, 1], f32, tag=tg + "ssum")
                    rinv = smalls.tile([C, 1], f32, tag=tg + "rinv")
                    nmx = smalls.tile([C, 1], f32, tag=tg + "nmx")
                    nc.vector.reduce_max(nmx, src_ps, axis=AX.X, negate=True)
                    nc.scalar.activation(out=dst, in_=src_ps, func=AF.Exp,
                                         bias=nmx, scale=1.0, accum_out=ssum)
                    nc.vector.reciprocal(rinv, ssum)
                    return rinv

                rinv2 = rowexp(S2_ps, ats_sb, "s2")
                rinv1 = rowexp(S1_ps, ast_sb, "s1")          # E1
                nc.vector.tensor_scalar_mul(ats_sb, ats_sb, rinv2)  # a_ts

                astT_ps = tl.tile([C, C], f16, tag="t1")
                atsT_ps = tl.tile([C, C], f16, tag="t2")
                nc.tensor.transpose(atsT_ps, ats_sb, ident_sb)
                nc.vector.tensor_copy(atsT_sb, atsT_ps)
                nc.tensor.transpose(astT_ps, ast_sb, ident_sb)
                nc.scalar.copy(astT_sb, astT_ps)

                m_ps = tl.tile([C, C], f32, tag="t3")
                nc.tensor.matmul(m_ps, lhsT=astT_sb, rhs=atsT_sb,
                                 start=True, stop=True)
                # m's entries are bounded in (0, 1], so no max-shift needed;
                # r1 rides the exp's per-row scale.
                msum = smalls.tile([C, 1], f32, tag="msum")
                mrinv = smalls.tile([C, 1], f32, tag="mrinv")
                nc.scalar.activation(out=att_sb, in_=m_ps, func=AF.Exp,
                                     scale=rinv1, accum_out=msum)
                nc.vector.reciprocal(mrinv, msum)
                nc.vector.tensor_scalar_mul(att_sb, att_sb, mrinv)

                attT_ps = tl.tile([C, C], f16, tag="t4")
                nc.tensor.transpose(attT_ps, att_sb, ident_sb)
                nc.vector.tensor_copy(attT_sb, attT_ps)

                # c = att vb ; A^T = vw^T att^T + I
                c_ps = tl.tile([C, 1], f32, tag="t2")
                nc.tensor.matmul(c_ps, lhsT=attT_sb, rhs=vb_sb,
                                 start=True, stop=True)
                nc.vector.tensor_copy(c_sb, c_ps)
                AT_ps = tl.tile([C, C], f32, tag="t1")
                nc.tensor.matmul(AT_ps, lhsT=vw_sb, rhs=attT_sb,
                                 start=True, stop=True)
                # fold the residual +I during the PSUM->SBUF copy
                nc.vector.tensor_add(AT_sb, AT_ps, ident_sb)

        # =========================== phase 2 ===========================
        # One matmul per 512-col chunk into 1024-col psum tiles; biased
        # copies (DVE/ACT alternating) gather into staging tiles; one SP
        # DMA per stage. Stage sizes ramp up so the first DMA fires early.
        STAGES = [1024, 1024] + [2048] * 6 + [1024, 1024]
        assert sum(STAGES) == n
        work = []
        lo = 0
        for w in STAGES:
            work.append((xs_sb, ys_d, slice(lo, lo + w)))
            work.append((xt_sb, yt_d, slice(lo, lo + w)))
            lo += w

        with tc.tile_pool(name="ops", bufs=4, space="PSUM") as o_ps_pool, \
             tc.tile_pool(name="osb", bufs=4) as o_sb_pool:
            eng = 0
            for idx, (x_sb, y_d, sl) in enumerate(work):
                stg = sl.stop - sl.start
                o_sb = o_sb_pool.tile([C, stg], f16, tag=f"o{stg}")
                for jj in range(max(1, stg // HP)):
                    hp = min(HP, stg)
                    o_ps = o_ps_pool.tile([C, HP], f32, tag="o")
                    for kk in range(hp // F2):
                        c0 = sl.start + jj * HP + kk * F2
                        csl = slice(c0, c0 + F2)
                        psl = slice(kk * F2, (kk + 1) * F2)
                        nc.tensor.matmul(o_ps[:, psl], lhsT=AT_sb,
                                         rhs=x_sb[:, csl],
                                         start=True, stop=True)
                    bsl = slice(jj * HP, jj * HP + hp)
                    # NOTE: GPSIMD cannot access PSUM on real HW, so the
                    # biased copies alternate DVE/ACT only.
                    if eng % 2 == 0:
                        nc.vector.tensor_scalar_add(o_sb[:, bsl],
                                                    o_ps[:, 0:hp], c_sb)
                    else:
                        nc.scalar.activation(out=o_sb[:, bsl],
                                             in_=o_ps[:, 0:hp],
                                             func=AF.Identity, bias=c_sb,
                                             scale=1.0)
                    eng += 1
                dma_eng = nc.sync if idx % 2 == 0 else nc.gpsimd
                dma_eng.dma_start(out=y_d[:, sl], in_=o_sb)

    nc.compile()
    return nc


def prep_core_inputs(x, qw, qb, kw, kb, vw, vb, n=N_FULL):
    """Build the 8 per-core input maps from full fp32 inputs."""
    f16 = np.float16
    wqk = np.ascontiguousarray(
        np.concatenate([qw.T, kw.T], axis=1)).astype(f16)
    vw_p = np.ascontiguousarray(vw).astype(f16)
    qb_row = qb.reshape(1, C).astype(f16)
    kb_row = kb.reshape(1, C).astype(f16)
    kbN_row = (float(n) * kb).reshape(1, C).astype(f16)
    vb_col = vb.reshape(C, 1).astype(f16)
    in_maps = []
    for i in range(8):
        in_maps.append({
            "xs": np.ascontiguousarray(x[i].reshape(C, n)).astype(f16),
            "xt": np.ascontiguousarray(x[i + 8].reshape(C, n)).astype(f16),
            "wqk": wqk,
            "vw_p": vw_p,
            "qb_row": qb_row,
            "kb_row": kb_row,
            "kbN_row": kbN_row,
            "vb_col": vb_col,
        })
    return in_maps


_NC_CACHE = {}


def get_nc():
    if "nc" not in _NC_CACHE:
        _NC_CACHE["nc"] = build_nc(N_FULL)
    return _NC_CACHE["nc"]


def run_device(x, qw, qb, kw, kb, vw, vb, trace=False):
    from concourse.bass_utils import run_bass_kernel_spmd

    nc = get_nc()
    in_maps = prep_core_inputs(x, qw, qb, kw, kb, vw, vb)
    res = run_bass_kernel_spmd(nc, in_maps, core_ids=list(range(8)),
                               trace=trace)
    y = np.empty((16, C, 128, 128), np.float32)
    for i in range(8):
        y[i] = res.results[i]["ys"].astype(np.float32).reshape(C, 128, 128)
        y[i + 8] = res.results[i]["yt"].astype(np.float32).reshape(C, 128, 128)
    return y, res


def kernel(**inputs):
    y, _ = run_device(
        np.asarray(inputs["x"]), np.asarray(inputs["qw"]),
        np.asarray(inputs["qb"]), np.asarray(inputs["kw"]),
        np.asarray(inputs["kb"]), np.asarray(inputs["vw"]),
        np.asarray(inputs["vb"]),
    )
    return y
